# revision 7
# baseline (speedup 1.0000x reference)
"""Ragged segment mean kernel for Trainium2 (8 NeuronCores, data-parallel).

Problem: seq [64, 2048, 1024] f32, begin/end [64] i64.
Output: out[i] = mean(seq[i, begin[i]:end[i], :])  -> [64, 1024] f32.

Strategy: dense-stream architecture. The host concatenates exactly the
segment rows of all samples into one global row stream, cuts it into 8
equal per-core chunks of R rows (R = ceil(total/8) rounded to 128), and
hands each core a contiguous [R, 1024] f32 buffer plus a per-128-row-
chunk one-hot routing mask. All device-side DMA offsets are then
compile-time constants: no runtime offset registers, no over-read, and
per-core load is balanced to the row.

On device each 128-row chunk is reduced on the PE as
acc[NSLOT, 512] += mask[128, NSLOT].T @ chunk[128, 512], accumulated in
PSUM over all chunks; the mask column routes each row to the slot
(= piece of a sample) it belongs to. Inputs are fed to the PE as
float32r via bitcast (1 cycle/row instead of fp32's 4): the f32r
mantissa rounding is ~1e-3 relative, far inside the 2e-2 gate, so no
hi/resid two-pass split is needed and the ACT/DVE engines stay idle.
The kernel is purely HBM-DMA-bound.

A sample whose rows straddle a core boundary becomes one piece per
core; each piece is scaled by the full 1/span on device and the host
adds the partial outputs while scattering back to batch order.
"""

import numpy as np

import concourse.bacc as bacc
import concourse.bass as bass
import concourse.mybir as mybir
import concourse.tile as tile
from concourse.bass_utils import run_bass_kernel_spmd

B, L, D = 64, 2048, 1024
NCORES = 8
FREE = 512              # PSUM bank limit for matmul free dim
NMM = D // FREE         # 2 matmuls per 128-row chunk
GROUP = 1024            # bulk rows per dma_start -> 4 MiB
NSLOT_DEFAULT = 16      # routing-mask columns (pieces per core)

_nc_cache = {}


def _group_sizes(R):
    """Split R rows into dma_start group sizes: 4 MiB bulk groups (one
    16-32 KiB descriptor per partition each keeps the SDMA engines
    streaming) with a 256+128-row tapered tail so the end-of-kernel
    DMA->PE drain is short."""
    sizes = []
    rem = R
    while rem >= GROUP + 384:
        sizes.append(GROUP)
        rem -= GROUP
    if rem > 384:
        sizes.append(rem - 384)
        rem = 384
    if rem == 384:
        sizes += [256, 128]
    elif rem:
        sizes.append(rem)
    return sizes


def _build_nc(R, NSLOT):
    nc = bacc.Bacc("TRN2", target_bir_lowering=False)
    f32 = mybir.dt.float32
    f32r = mybir.dt.float32r
    NCH = R // 128
    # seq and mask are declared float32r end-to-end (same 4-byte storage;
    # the host supplies raw fp32 bits). The PE's f32r mode rounds inputs
    # internally (~1e-4 rel) and runs 1 cycle/row instead of fp32's 4;
    # feeding it straight from DMA keeps ACT/DVE completely idle.
    seqc = nc.dram_tensor("seqc", [R, D], f32r, kind="ExternalInput")
    maskt = nc.dram_tensor("maskt", [128, NCH * NSLOT], f32r, kind="ExternalInput")
    invc = nc.dram_tensor("invc", [NSLOT, 1], f32, kind="ExternalInput")
    out = nc.dram_tensor("out", [NSLOT, D], f32, kind="ExternalOutput")

    sizes = _group_sizes(R)
    GF = (GROUP // 128) * D  # free size of a full group tile

    with tile.TileContext(nc) as tc:
        with (
            tc.tile_pool(name="const", bufs=1) as cpool,
            tc.tile_pool(name="seqp", bufs=4) as spool,
            tc.tile_pool(name="accp", bufs=1, space="PSUM") as ppool,
            tc.tile_pool(name="resp", bufs=1) as rpool,
        ):
            mt = cpool.tile([128, NCH * NSLOT], f32r)
            iv = cpool.tile([NSLOT, 1], f32)
            iv2 = cpool.tile([NSLOT, 1], f32, tag="iv2")
            warm = ppool.tile([NSLOT, NSLOT], f32, tag="warm")
            acc = ppool.tile([NSLOT, D], f32)

            # The mask rides FIRST on the same HWDGE ring/queue as the seq
            # stream: FIFO order lands it before group 0 completes, so the
            # PE pipeline starts ~8us in. (On the other ring it round-robins
            # against the stream packet-by-packet and arrives ~19us late,
            # stalling every matmul behind it.)
            nc.sync.dma_start(out=mt[:], in_=maskt[:])
            # iv is only needed by the final scale; keep its tiny DMA off
            # the stream queue.
            nc.scalar.dma_start(out=iv[:], in_=invc[:])
            nc.vector.tensor_copy(out=iv2[:], in_=iv[:])

            g0 = 0  # stream row base of current group
            ch = 0  # global 128-row chunk counter
            for gi, U in enumerate(sizes):
                J = U // 128
                gf = J * D
                t = spool.tile([128, GF], f32r)
                # group tile[p, j*D+d] holds stream row g0 + p*J + j:
                # partition p reads J*4KiB contiguous from DRAM.
                src = seqc[g0 : g0 + U, :].rearrange("(p j) d -> p (j d)", p=128)
                nc.sync.dma_start(out=t[:, 0:gf], in_=src)
                if gi == 0:
                    # warmup matmul consuming only the mask tile: absorbs
                    # the mask-arrival dependency into the PE clock so the
                    # first real matmul waits only on the seq pipeline.
                    nc.tensor.matmul(
                        out=warm[:],
                        lhsT=mt[:, 0:NSLOT],
                        rhs=mt[:, 0:NSLOT],
                        start=True,
                        stop=True,
                    )
                for j in range(J):
                    lhs = mt[:, (ch + j) * NSLOT : (ch + j + 1) * NSLOT]
                    for h in range(NMM):
                        nc.tensor.matmul(
                            out=acc[:, h * FREE : (h + 1) * FREE],
                            lhsT=lhs,
                            rhs=t[:, j * D + h * FREE : j * D + (h + 1) * FREE],
                            start=(ch + j == 0),
                            stop=(ch + j == NCH - 1),
                        )
                ch += J
                g0 += U

            res = rpool.tile([NSLOT, D], f32)
            nc.vector.tensor_scalar_mul(out=res[:], in0=acc[:], scalar1=iv2[:])
            nc.sync.dma_start(out=out[:], in_=res[:])
    nc.compile()
    return nc


def _plan(begin, end):
    """Order samples (big/small interleave to bound pieces per core),
    cut the global segment-row stream into 8 R-row cores, and return
    (R, NSLOT, pieces) with pieces[ci] = [(sample, src_b, src_e, local
    row start), ...]."""
    spans = (end - begin).astype(np.int64)
    desc = np.argsort(-spans, kind="stable")
    order = np.empty(B, dtype=np.int64)
    order[0::2] = desc[: (B + 1) // 2]
    order[1::2] = desc[(B + 1) // 2 :][::-1]
    total = int(spans.sum())
    per_core = -(-total // NCORES)          # ceil(total / 8)
    R = -(-per_core // 128) * 128           # round up to 128 rows

    pieces = [[] for _ in range(NCORES)]
    g = 0  # global stream cursor
    for i in order:
        b, e = int(begin[i]), int(end[i])
        while b < e:
            ci = g // R
            room = (ci + 1) * R - g
            n = min(e - b, room)
            pieces[ci].append((int(i), b, b + n, g - ci * R))
            b += n
            g += n
    nslot = max(NSLOT_DEFAULT, max(len(p) for p in pieces))
    nslot = -(-nslot // 8) * 8
    return R, nslot, pieces


def _make_in_maps(seq, pieces, spans, R, NSLOT):
    NCH = R // 128
    in_maps = []
    for ci in range(NCORES):
        seqc = np.zeros((R, D), dtype=np.float32)
        row_slot = np.full(R, -1, dtype=np.int64)
        inv = np.zeros((NSLOT, 1), dtype=np.float32)
        for s, (i, sb, se, ls) in enumerate(pieces[ci]):
            n = se - sb
            seqc[ls : ls + n] = seq[i, sb:se]
            row_slot[ls : ls + n] = s
            inv[s, 0] = np.float32(1.0 / float(spans[i]))
        mt = np.zeros((128, NCH * NSLOT), dtype=np.float32)
        slot_ids = np.arange(NSLOT)
        g0 = 0
        chb = 0
        for U in _group_sizes(R):
            J = U // 128
            for j in range(J):
                rows = row_slot[g0 + np.arange(128) * J + j]
                oh = (rows[:, None] == slot_ids[None, :]).astype(np.float32)
                mt[:, (chb + j) * NSLOT : (chb + j + 1) * NSLOT] = oh
            g0 += U
            chb += J
        in_maps.append({"seqc": seqc, "maskt": mt, "invc": inv})
    return in_maps


def _axon_reset():
    """Best-effort NeuronCore reset (recovers a device wedged by an
    earlier failed run in the same container)."""
    try:
        import ctypes

        import jax

        jax.devices()
        lib = ctypes.CDLL("/opt/axon/libaxon_pjrt.so")
        lib.axon_reset.restype = ctypes.c_int64
        lib.axon_reset()
    except Exception:
        pass


def _run(seq, begin, end, trace=False):
    seq = np.asarray(seq)
    begin = np.asarray(begin).astype(np.int64)
    end = np.asarray(end).astype(np.int64)
    spans = end - begin
    R, NSLOT, pieces = _plan(begin, end)
    key = (R, NSLOT)
    if key not in _nc_cache:
        _nc_cache[key] = _build_nc(R, NSLOT)
    in_maps = _make_in_maps(seq, pieces, spans, R, NSLOT)
    try:
        res = run_bass_kernel_spmd(
            _nc_cache[key], in_maps, list(range(NCORES)), trace=trace
        )
    except Exception:
        _axon_reset()
        res = run_bass_kernel_spmd(
            _nc_cache[key], in_maps, list(range(NCORES)), trace=trace
        )
    out = np.zeros((B, D), dtype=np.float32)
    for ci in range(NCORES):
        part = res.results[ci]["out"]
        for s, (i, sb, se, ls) in enumerate(pieces[ci]):
            out[i] += part[s]
    return out, res


def kernel(seq, begin, end):
    out, _ = _run(seq, begin, end, trace=False)
    return out


# revision 10
# speedup vs baseline: 1.1418x; 1.1418x over previous
"""Ragged segment mean kernel for Trainium2 (8 NeuronCores, data-parallel).

Problem: seq [64, 2048, 1024] f32, begin/end [64] i64.
Output: out[i] = mean(seq[i, begin[i]:end[i], :])  -> [64, 1024] f32.

Strategy: dense-stream architecture. The host concatenates exactly the
segment rows of all samples into one global row stream, cuts it into 8
equal per-core chunks of R rows (R = ceil(total/8) rounded to 128), and
hands each core a contiguous [R, 1024] f32 buffer plus a per-128-row-
chunk one-hot routing mask. All device-side DMA offsets are then
compile-time constants: no runtime offset registers, no over-read, and
per-core load is balanced to the row.

On device each 128-row chunk is reduced on the PE as
acc[NSLOT, 512] += mask[128, NSLOT].T @ chunk[128, 512], accumulated in
PSUM over all chunks; the mask column routes each row to the slot
(= piece of a sample) it belongs to. Inputs are fed to the PE as
float32r via bitcast (1 cycle/row instead of fp32's 4): the f32r
mantissa rounding is ~1e-3 relative, far inside the 2e-2 gate, so no
hi/resid two-pass split is needed and the ACT/DVE engines stay idle.
The kernel is purely HBM-DMA-bound.

A sample whose rows straddle a core boundary becomes one piece per
core; each piece is scaled by the full 1/span on device and the host
adds the partial outputs while scattering back to batch order.
"""

import numpy as np

import concourse.bacc as bacc
import concourse.bass as bass
import concourse.mybir as mybir
import concourse.tile as tile
from concourse.bass_utils import run_bass_kernel_spmd

B, L, D = 64, 2048, 1024
NCORES = 8
FREE = 512              # PSUM bank limit for matmul free dim
NMM = D // FREE         # 2 matmuls per 128-row chunk
GROUP = 512             # bulk rows per dma_start -> 2 MiB
NSLOT_DEFAULT = 16      # routing-mask columns (pieces per core)

_nc_cache = {}


def _group_sizes(R):
    """Split R rows into dma_start group sizes: 2 MiB bulk groups (16 KiB
    contiguous per partition each) with a 256+128-row tapered tail so the
    end-of-kernel DMA->PE drain is short. Groups stay smallish because a
    matmul can only start once its whole group's DMA semaphore fires:
    4 MiB groups were measured to turn the PE into a bursty straggler
    that outlives the stream by >10us."""
    sizes = []
    rem = R
    while rem >= GROUP + 384:
        sizes.append(GROUP)
        rem -= GROUP
    if rem > 384:
        sizes.append(rem - 384)
        rem = 384
    if rem == 384:
        sizes += [256, 128]
    elif rem:
        sizes.append(rem)
    return sizes


def _build_nc(R, NSLOT):
    nc = bacc.Bacc("TRN2", target_bir_lowering=False)
    f32 = mybir.dt.float32
    f32r = mybir.dt.float32r
    NCH = R // 128
    # seq and mask are declared float32r end-to-end (same 4-byte storage;
    # the host supplies raw fp32 bits). The PE's f32r mode rounds inputs
    # internally (~1e-4 rel) and runs 1 cycle/row instead of fp32's 4;
    # feeding it straight from DMA keeps ACT/DVE completely idle.
    seqc = nc.dram_tensor("seqc", [R, D], f32r, kind="ExternalInput")
    maskt = nc.dram_tensor("maskt", [128, NCH * NSLOT], f32r, kind="ExternalInput")
    invc = nc.dram_tensor("invc", [NSLOT, 1], f32, kind="ExternalInput")
    out = nc.dram_tensor("out", [NSLOT, D], f32, kind="ExternalOutput")

    sizes = _group_sizes(R)
    GF = (GROUP // 128) * D  # free size of a full group tile

    with tile.TileContext(nc) as tc:
        with (
            tc.tile_pool(name="const", bufs=1) as cpool,
            tc.tile_pool(name="seqp", bufs=8) as spool,
            tc.tile_pool(name="accp", bufs=1, space="PSUM") as ppool,
            tc.tile_pool(name="resp", bufs=1) as rpool,
        ):
            mt = cpool.tile([128, NCH * NSLOT], f32r)
            iv = cpool.tile([NSLOT, 1], f32)
            iv2 = cpool.tile([NSLOT, 1], f32, tag="iv2")
            warm = ppool.tile([NSLOT, NSLOT], f32, tag="warm")
            acc = ppool.tile([NSLOT, D], f32)

            # The mask rides FIRST on the same HWDGE ring/queue as the seq
            # stream: FIFO order lands it before group 0 completes, so the
            # PE pipeline starts ~8us in. (On the other ring it round-robins
            # against the stream packet-by-packet and arrives ~19us late,
            # stalling every matmul behind it.)
            nc.sync.dma_start(out=mt[:], in_=maskt[:])
            # iv is only needed by the final scale; keep its tiny DMA off
            # the stream queue.
            nc.scalar.dma_start(out=iv[:], in_=invc[:])
            nc.vector.tensor_copy(out=iv2[:], in_=iv[:])

            g0 = 0  # stream row base of current group
            ch = 0  # global 128-row chunk counter
            for gi, U in enumerate(sizes):
                J = U // 128
                gf = J * D
                t = spool.tile([128, GF], f32r)
                # group tile[p, j*D+d] holds stream row g0 + p*J + j:
                # partition p reads J*4KiB contiguous from DRAM. Groups
                # alternate between the two HWDGE rings (SP and ACT) so
                # per-dma queue gaps on one ring hide under the other's
                # stream and the SDMA engines never starve.
                src = seqc[g0 : g0 + U, :].rearrange("(p j) d -> p (j d)", p=128)
                ring = nc.sync if gi % 2 == 0 else nc.scalar
                ring.dma_start(out=t[:, 0:gf], in_=src)
                if gi == 0:
                    # warmup matmul consuming only the mask tile: absorbs
                    # the mask-arrival dependency into the PE clock so the
                    # first real matmul waits only on the seq pipeline.
                    nc.tensor.matmul(
                        out=warm[:],
                        lhsT=mt[:, 0:NSLOT],
                        rhs=mt[:, 0:NSLOT],
                        start=True,
                        stop=True,
                    )
                for j in range(J):
                    lhs = mt[:, (ch + j) * NSLOT : (ch + j + 1) * NSLOT]
                    for h in range(NMM):
                        nc.tensor.matmul(
                            out=acc[:, h * FREE : (h + 1) * FREE],
                            lhsT=lhs,
                            rhs=t[:, j * D + h * FREE : j * D + (h + 1) * FREE],
                            start=(ch + j == 0),
                            stop=(ch + j == NCH - 1),
                        )
                ch += J
                g0 += U

            res = rpool.tile([NSLOT, D], f32)
            nc.vector.tensor_scalar_mul(out=res[:], in0=acc[:], scalar1=iv2[:])
            nc.sync.dma_start(out=out[:], in_=res[:])
    nc.compile()
    return nc


def _plan(begin, end):
    """Order samples (big/small interleave to bound pieces per core),
    cut the global segment-row stream into 8 R-row cores, and return
    (R, NSLOT, pieces) with pieces[ci] = [(sample, src_b, src_e, local
    row start), ...]."""
    spans = (end - begin).astype(np.int64)
    desc = np.argsort(-spans, kind="stable")
    order = np.empty(B, dtype=np.int64)
    order[0::2] = desc[: (B + 1) // 2]
    order[1::2] = desc[(B + 1) // 2 :][::-1]
    total = int(spans.sum())
    per_core = -(-total // NCORES)          # ceil(total / 8)
    R = -(-per_core // 128) * 128           # round up to 128 rows

    pieces = [[] for _ in range(NCORES)]
    g = 0  # global stream cursor
    for i in order:
        b, e = int(begin[i]), int(end[i])
        while b < e:
            ci = g // R
            room = (ci + 1) * R - g
            n = min(e - b, room)
            pieces[ci].append((int(i), b, b + n, g - ci * R))
            b += n
            g += n
    nslot = max(NSLOT_DEFAULT, max(len(p) for p in pieces))
    nslot = -(-nslot // 8) * 8
    return R, nslot, pieces


def _make_in_maps(seq, pieces, spans, R, NSLOT):
    NCH = R // 128
    in_maps = []
    for ci in range(NCORES):
        seqc = np.zeros((R, D), dtype=np.float32)
        row_slot = np.full(R, -1, dtype=np.int64)
        inv = np.zeros((NSLOT, 1), dtype=np.float32)
        for s, (i, sb, se, ls) in enumerate(pieces[ci]):
            n = se - sb
            seqc[ls : ls + n] = seq[i, sb:se]
            row_slot[ls : ls + n] = s
            inv[s, 0] = np.float32(1.0 / float(spans[i]))
        mt = np.zeros((128, NCH * NSLOT), dtype=np.float32)
        slot_ids = np.arange(NSLOT)
        g0 = 0
        chb = 0
        for U in _group_sizes(R):
            J = U // 128
            for j in range(J):
                rows = row_slot[g0 + np.arange(128) * J + j]
                oh = (rows[:, None] == slot_ids[None, :]).astype(np.float32)
                mt[:, (chb + j) * NSLOT : (chb + j + 1) * NSLOT] = oh
            g0 += U
            chb += J
        in_maps.append({"seqc": seqc, "maskt": mt, "invc": inv})
    return in_maps


def _axon_reset():
    """Best-effort NeuronCore reset (recovers a device wedged by an
    earlier failed run in the same container)."""
    try:
        import ctypes

        import jax

        jax.devices()
        lib = ctypes.CDLL("/opt/axon/libaxon_pjrt.so")
        lib.axon_reset.restype = ctypes.c_int64
        lib.axon_reset()
    except Exception:
        pass


def _run(seq, begin, end, trace=False):
    seq = np.asarray(seq)
    begin = np.asarray(begin).astype(np.int64)
    end = np.asarray(end).astype(np.int64)
    spans = end - begin
    R, NSLOT, pieces = _plan(begin, end)
    key = (R, NSLOT)
    if key not in _nc_cache:
        _nc_cache[key] = _build_nc(R, NSLOT)
    in_maps = _make_in_maps(seq, pieces, spans, R, NSLOT)
    try:
        res = run_bass_kernel_spmd(
            _nc_cache[key], in_maps, list(range(NCORES)), trace=trace
        )
    except Exception:
        _axon_reset()
        res = run_bass_kernel_spmd(
            _nc_cache[key], in_maps, list(range(NCORES)), trace=trace
        )
    out = np.zeros((B, D), dtype=np.float32)
    for ci in range(NCORES):
        part = res.results[ci]["out"]
        for s, (i, sb, se, ls) in enumerate(pieces[ci]):
            out[i] += part[s]
    return out, res


def kernel(seq, begin, end):
    out, _ = _run(seq, begin, end, trace=False)
    return out
